# revision 6
# baseline (speedup 1.0000x reference)
"""PointClickLoss Trainium2 kernel.

Reference computes, for each of 32 images (1024x1024), bilinear samples at
16 positive + 16 negative points, BCE-with-logits losses via softplus, and
a mean over points/images.  Only 4 taps per point are actually needed from
the 128 MB pred_mask, so the kernel is a tiny data-dependent gather:

  - data parallel over 8 NeuronCores, 4 images each
  - per core: 128 points -> one partition each (pos points in partitions
    0..63, neg in 64..127; host concatenates the two point tensors)
  - tap indices computed on-device (frac/floor via DVE mod), gathered with
    one indirect DMA (256 descriptors x 8 bytes: two x-adjacent taps per
    descriptor, two row descriptors per point)
  - bilinear combine + softplus(+/-z)=ln(1+exp(+/-z)) on DVE/ACT,
    partition-sum via PE matmul with a ones vector
  - each core emits its partial sum / 1024; host adds the 8 scalars
"""

import sys

if "/opt/trn_rl_repo" not in sys.path:
    sys.path.insert(0, "/opt/trn_rl_repo")

import numpy as np

B, H, W = 32, 1024, 1024
NPOS = NNEG = 16
NCORES = 8
BL = B // NCORES          # images per core
P = 128                   # partitions used = BL * (NPOS + NNEG)

_BUILT = None
LAST_RESULTS = None       # BassKernelResults of the most recent run (for test.py)


def build():
    import concourse.bass as bass
    import concourse.mybir as mybir
    import concourse.tile as tile
    from concourse import bacc

    f32 = mybir.dt.float32
    i32 = mybir.dt.int32
    mult = mybir.AluOpType.mult
    add = mybir.AluOpType.add
    bypass = mybir.AluOpType.bypass

    nc = bacc.Bacc()
    pm = nc.dram_tensor("pm", [BL * H * W, 1], f32, kind="ExternalInput")
    pts_d = nc.dram_tensor("pts", [P, 2], f32, kind="ExternalInput")
    out_d = nc.dram_tensor("out", [1, 1], f32, kind="ExternalOutput")

    # Per-partition constants: partition p < 64 is positive point (p % 16) of
    # image p // 16; partition p >= 64 is negative point of image (p-64)//16.
    # cols: [row0 base, row1 base, softplus sign, one, zero]
    pidx = np.arange(P)
    img = (pidx % 64) // 16
    base = (img * H * W).astype(np.float64)
    cb_np = np.stack(
        [
            base,
            base + W,
            np.where(pidx < 64, -1.0, 1.0),
            np.ones(P),
            np.zeros(P),
        ],
        axis=1,
    ).astype(np.float32)
    cb_d = nc.inline_tensor(cb_np, name="cb_const")

    with tile.TileContext(nc) as tc:
        with (
            tc.tile_pool(name="sb", bufs=1) as pool,
            tc.tile_pool(name="ps", bufs=1, space="PSUM") as psum,
        ):
            pts = pool.tile([P, 2], f32)          # (x, y) per point
            nc.sync.dma_start(pts[:, :], pts_d[:, :])
            cb = pool.tile([P, 5], f32)
            nc.sync.dma_start(cb[:], cb_d[:, :])

            # floor/frac; points are in [0, 1023) so floor(x) <= 1022 and
            # x0+1 is always in range.  floor = roundtrip through int32 with
            # a compare-correct, valid for truncating or nearest conversions.
            xi = pool.tile([P, 2], i32)
            nc.vector.tensor_copy(xi[:], pts[:])
            xf = pool.tile([P, 2], f32)
            nc.vector.tensor_copy(xf[:], xi[:])
            dgt = pool.tile([P, 2], f32)          # 1.0 where rounded up
            nc.vector.tensor_tensor(dgt[:], xf[:], pts[:], mybir.AluOpType.is_gt)
            xy0 = pool.tile([P, 2], f32)          # (x0, y0) as floats (exact)
            nc.vector.tensor_sub(xy0[:], xf[:], dgt[:])
            fr = pool.tile([P, 2], f32)           # (wx1, wy1)
            nc.vector.tensor_sub(fr[:], pts[:], xy0[:])
            g1 = pool.tile([P, 2], f32)           # (wx0, wy0) = 1 - frac
            nc.vector.tensor_scalar(
                out=g1[:], in0=fr[:], scalar1=-1.0, scalar2=1.0,
                op0=mult, op1=add,
            )

            # Flat element index of the (y0, x0) tap, then +base / +base+W for
            # the two row taps.  All values < 2^23 so f32 arithmetic is exact.
            r = pool.tile([P, 1], f32)
            nc.vector.scalar_tensor_tensor(
                out=r[:], in0=xy0[:, 1:2], scalar=float(W), in1=xy0[:, 0:1],
                op0=mult, op1=add,
            )
            idxf = pool.tile([P, 2], f32)
            nc.vector.scalar_tensor_tensor(
                out=idxf[:], in0=cb[:, 0:2], scalar=r[:, 0:1], in1=cb[:, 0:2],
                op0=add, op1=bypass,
            )
            idx = pool.tile([P, 2], i32)
            nc.vector.tensor_copy(idx[:], idxf[:])

            # Gather the 2x2 taps: descriptor (p, j) reads 2 contiguous f32 at
            # pm[idx[p, j]] into gv[p, 2j:2j+2]  ->  gv = (v00, v01, v10, v11).
            gv = pool.tile([P, 4], f32)
            nc.gpsimd.indirect_dma_start(
                out=gv[:],
                out_offset=None,
                in_=pm[:, :],
                in_offset=bass.IndirectOffsetOnAxis(ap=idx[:, :], axis=0),
            )

            # Bilinear weights (wy0*wx0, wy0*wx1, wy1*wx0, wy1*wx1)
            w4 = pool.tile([P, 4], f32)
            nc.vector.tensor_tensor(w4[:, 0:1], g1[:, 1:2], g1[:, 0:1], mult)
            nc.vector.tensor_tensor(w4[:, 1:2], g1[:, 1:2], fr[:, 0:1], mult)
            nc.vector.tensor_tensor(w4[:, 2:3], fr[:, 1:2], g1[:, 0:1], mult)
            nc.vector.tensor_tensor(w4[:, 3:4], fr[:, 1:2], fr[:, 0:1], mult)

            # val[p] = sum_j gv[p, j] * w4[p, j]
            tt = pool.tile([P, 4], f32)
            val = pool.tile([P, 1], f32)
            nc.vector.scalar_tensor_tensor(
                out=tt[:], in0=gv[:], scalar=1.0, in1=w4[:],
                op0=bypass, op1=mult, accum_out=val[:],
            )

            # loss[p] = softplus(sign[p] * val[p]) = ln(1 + exp(sign*val)).
            # bias/scale come from cb columns so no extra const DMAs appear.
            ez = pool.tile([P, 1], f32)
            nc.scalar.activation(
                out=ez[:], in_=val[:],
                func=mybir.ActivationFunctionType.Exp,
                bias=cb[:, 4:5], scale=cb[:, 2:3],
            )
            sp = pool.tile([P, 1], f32)
            nc.scalar.activation(
                out=sp[:], in_=ez[:],
                func=mybir.ActivationFunctionType.Ln,
                bias=cb[:, 3:4],
            )

            # partition sum via ones-vector matmul, scaled by 1/(B*NPTS)
            acc = psum.tile([1, 1], f32)
            nc.tensor.matmul(
                out=acc[:], lhsT=sp[:], rhs=cb[:, 3:4], start=True, stop=True
            )
            res = pool.tile([1, 1], f32)
            nc.scalar.mul(res[:], acc[:], 1.0 / float(B * (NPOS + NNEG)))
            nc.sync.dma_start(out_d[:, :], res[:])

    nc.compile()
    return nc


def shard_inputs(pred_mask, positive_points, negative_points):
    pm = np.ascontiguousarray(
        np.asarray(pred_mask, dtype=np.float32).reshape(NCORES, BL * H * W, 1)
    )
    pos = np.asarray(positive_points, dtype=np.float32).reshape(NCORES, BL * NPOS, 2)
    neg = np.asarray(negative_points, dtype=np.float32).reshape(NCORES, BL * NNEG, 2)
    pts = np.ascontiguousarray(np.concatenate([pos, neg], axis=1))  # [8, 128, 2]
    return [{"pm": pm[c], "pts": pts[c]} for c in range(NCORES)]


def kernel(pred_mask, positive_points, negative_points):
    global _BUILT, LAST_RESULTS
    from concourse.bass_utils import run_bass_kernel_spmd

    if _BUILT is None:
        _BUILT = build()
    in_maps = shard_inputs(pred_mask, positive_points, negative_points)
    res = run_bass_kernel_spmd(_BUILT, in_maps, core_ids=list(range(NCORES)))
    LAST_RESULTS = res
    total = float(sum(float(r["out"][0, 0]) for r in res.results))
    return np.float32(total)
